# revision 39
# baseline (speedup 1.0000x reference)
"""MemoryBank.get_all_distances Trainium2 kernel (v2).

emb_batch [64, 64] f32, bank [131072, 64] f32 -> distances [64, 131072] f32
  distances[n, b] = || bank[b] - emb[n] ||_2

Strategy: shard bank rows across 8 cores (16384 rows each); all heavy
arithmetic on device:

  dist^2[n, b] = (||e_n||^2 + 1) - 2 e_n . b_b        (bank rows unit-norm)

v2 changes vs the 27.2us baseline (all motivated by its perfetto trace):
 - bank ships as fp8 e3m4 scaled x8 (1MB/core instead of 2MB bf16): the
   input DMA stream was a main serialization. The x8 scale keeps bank
   values in e3m4's normal range; the 1/8 is folded into the stationary.
 - the serial ACT-sqrt chain (9.8us) is split ~50/50 with the Vector
   engine via a custom DVE op SQRT_DIST_ANT: a single 8-stage uop
   computing (t*w)*(c - t*w^2), w = a - t, t = psum + bias -- a
   seed+Newton sqrt whose constants are fitted PER QUERY (the [P,1]
   scalar slots are per-partition APs). Each query's dist^2 range is
   [(|e|-1)^2, (|e|+1)^2] (Cauchy-Schwarz, unit bank rows), a ~1.5-1.9x
   ratio, where the fitted family is accurate to <1e-3. The per-query
   input scale k_n (the fit's T = k*t normalization) is folded into the
   stationary column and bias, so the op needs only 3 runtime constants
   (bias, a, c), all riding an extended ewc tensor. The ACT half uses
   the exact table Sqrt with scale=1/k (per-partition AP) to undo k.
 - PE warm-up extended to N_WARM back-to-back matmuls so the PE p-state
   ramp (0.65 -> 1.2 -> 2.4 GHz with continuous execution) is not reset
   by the idle gap waiting for the first bank chunk.
 - inputs ride all four DMA-capable engine queues (sync/scalar/gpsimd/
   vector); outputs pair 1024-col tiles on gpsimd/sync, and the last
   pair splits into two 512-col DMAs issued by scalar/vector right
   after each computes its half, shortening the drain tail.

Host-side work is re-layout, casts, and O(batch) per-query constant
algebra; the 64 x 131072 distance computation runs on device.
"""

import numpy as np

BANK = 131072
DIM = 64
BATCH = 64
N_CORES = 8
SHARD = BANK // N_CORES  # 16384 bank rows per core
HALF = SHARD // 2  # 8192 columns per partition-half
NBLK = 512  # psum bank / matmul block
NB = HALF // NBLK  # 16 blocks
N_WARM = 12  # PE warm-up dummy matmuls (bridge ~7.7us to first data ~11.4us)
WARM_W = 384

EWC_COLS = 264  # 0:128 tiled -2*embT (ACT stationary), 128:256 tiled
#                 -embT*k/4 (DVE stationary), 256 bias_act, 257 bias_T,
#                 258 a, 259 c, 260:264 pad

# input DMA regions: (engine, col_start, width). Few large transfers:
# each extra DMA on a queue costs ~1.7-2.5us of completion overhead,
# so one small head chunk + one big chain per queue beats many pairs.
IN_DMA = [
    ("sync", 0, 1024),
    ("sync", 1024, 2048),
    ("scalar", 3072, 2560),
    ("gpsimd", 5632, 2560),
]
# processing units in order: (col_start, width, consumer, out engine).
# ACT (1.09ns/col) and DVE (1.31ns/col) stream concurrently; each unit
# ships its own output DMA as soon as its sqrt completes (receipts
# pipeline during compute). gpsimd ships the early units (its exit
# DRAIN waits its own last receipt), sync mid, scalar the last.
UNITS = [
    (0, 512, "act", "gpsimd"),
    (512, 512, "dve", "gpsimd"),
    (5632, 1024, "dve", "gpsimd"),
    (6656, 1024, "act", "sync"),
    (7680, 512, "act", "sync"),
    (3072, 1024, "act", "sync"),
    (4096, 1024, "dve", "sync"),
    (5120, 512, "act", "scalar"),
    (1024, 1024, "dve", "sync"),
    (2048, 1024, "act", "scalar"),
]

# (rho, A, C, D): minimax fit of (u(A-u))*(C - D*u*(A-u)^2) ~ sqrt(u) on
# [1, rho]. Runtime picks the first row with rho >= hi/lo per query.
RHO_GRID = np.array([
    [1.050000, 3.100072455e+00, 7.141199749e-01, 5.395227285e-02],
    [1.100000, 3.200621996e+00, 6.811122775e-01, 4.681143816e-02],
    [1.150000, 3.300036688e+00, 6.511194075e-01, 4.089558971e-02],
    [1.200000, 3.399225914e+00, 6.235142256e-01, 3.591125361e-02],
    [1.250000, 3.501101816e+00, 5.973067924e-01, 3.157023771e-02],
    [1.300000, 3.599322163e+00, 5.738844723e-01, 2.799945435e-02],
    [1.350000, 3.701183808e+00, 5.513019518e-01, 2.482163750e-02],
    [1.400000, 3.789154066e+00, 5.330612181e-01, 2.243760216e-02],
    [1.450000, 3.879126690e+00, 5.155052690e-01, 2.029188674e-02],
    [1.500000, 3.990011604e+00, 4.952743372e-01, 1.799393853e-02],
    [1.550000, 4.076589519e+00, 4.804826293e-01, 1.642816584e-02],
    [1.600000, 4.166412065e+00, 4.659125308e-01, 1.497714111e-02],
    [1.650000, 4.301812211e+00, 4.454678765e-01, 1.308861788e-02],
    [1.700000, 4.011049261e+00, 4.916104814e-01, 1.759726228e-02],
    [1.750000, 4.463828625e+00, 4.230602914e-01, 1.120840452e-02],
    [1.800000, 4.550521068e+00, 4.118879986e-01, 1.034208225e-02],
    [1.850000, 4.643520730e+00, 4.004849308e-01, 9.504965304e-03],
    [1.900000, 4.720735273e+00, 3.914057772e-01, 8.871692706e-03],
    [1.950000, 4.797335810e+00, 3.828324900e-01, 8.299883685e-03],
    [2.000000, 4.853129788e+00, 3.767461691e-01, 7.909282946e-03],
    [2.050000, 5.094606393e+00, 3.523314527e-01, 6.464805991e-03],
    [2.100000, 5.154723914e+00, 3.467030243e-01, 6.158776046e-03],
    [2.150000, 5.213427650e+00, 3.413737054e-01, 5.878041725e-03],
    [2.200000, 5.264111260e+00, 3.369582542e-01, 5.652122092e-03],
    [2.250000, 5.316573553e+00, 3.324348601e-01, 5.426577222e-03],
    [2.300000, 5.527506900e+00, 3.151706598e-01, 4.620604985e-03],
    [2.350000, 5.628931061e+00, 3.074716570e-01, 4.288479156e-03],
    [2.400000, 5.440198415e+00, 3.221203523e-01, 4.934757734e-03],
    [2.450000, 5.791933067e+00, 2.957967245e-01, 3.815696962e-03],
    [2.500000, 5.663749996e+00, 3.050162576e-01, 4.186206619e-03],
    [2.550000, 5.961363104e+00, 2.844794628e-01, 3.391697650e-03],
    [2.600000, 6.104152735e+00, 2.750444803e-01, 3.062144227e-03],
    [2.700000, 6.334969116e+00, 2.623556616e-01, 2.655511356e-03],
    [2.800000, 6.424367652e+00, 2.572218419e-01, 2.501381920e-03],
    [2.900000, 6.534776213e+00, 2.515605932e-01, 2.338734711e-03],
    [3.000000, 6.566494359e+00, 2.499358968e-01, 2.293314916e-03],
])

_cache = {}

# test.py reads this after calling kernel() to get profiling info.
last_run = None


def _ensure_dve_op():
    """Register the fused sqrt custom-DVE op (idempotent).

    out = (t*w) * (Src1 - t*w^2), w = C1 - t, t = Src0 + C0. C0 (bias)
    and C1 (a) ride s0/s1 as per-partition [P,1] APs; c rides Src1 as a
    zero-stride broadcast stream (GPSIMD cannot read PSUM on TRN2, so
    the bias-add must stay inside this op). ONE uop, 7 stages + bypass;
    measured ~2 cycles/elem on HW (PSUM source + second stream).
    """
    if "dve_op" in _cache:
        return _cache["dve_op"]
    import numpy as _np
    from concourse import dve_ops
    from concourse.dve_spec import C0, C1, Spec, Src0, Src1

    def _ref(in0, in1, s0, s1, imm2):
        t = (_np.asarray(in0, _np.float32) + s0).astype(_np.float32)
        w = (s1 - t).astype(_np.float32)
        g = (in1 - (t * (w * w)).astype(_np.float32)).astype(_np.float32)
        return ((t * w).astype(_np.float32) * g).astype(_np.float32)

    t = Src0 + C0
    w = C1 - t
    spec = Spec(body=(t * w) * (Src1 - t * (w * w)), reference=_ref)
    op = dve_ops.DveOp(
        "SQRT_DIST_ANT",
        spec,
        subdim=False,
        uops_sha={"v3": "557844d01f1a7ca6", "v4": "8b7c8b0259b84c2e"},
    )
    if op.name not in dve_ops._SUB_OPCODE_FOR_NAME:
        row = max(dve_ops._SUB_OPCODE_FOR_NAME.values()) + 1
        assert row < 0x20, "custom-DVE opcode rows exhausted"
        dve_ops.OPS.append(op)
        dve_ops.CUSTOM_DVE_SPECS[op.name] = op.spec
        dve_ops._SUB_OPCODE_FOR_NAME[op.name] = row
    _cache["dve_op"] = op
    return op


def _maybe_enable_ldw_opt():
    """Experiment hook: walrus is invoked with --enable-ldw-opt=false;
    flipping it lets codegen elide back-to-back LDWEIGHTS of the same
    stationary (~130ns of PE per matmul here). Gated on KV2_LDWOPT=1."""
    import os

    if os.environ.get("KV2_LDWOPT", "0") != "1" or "ldwopt" in _cache:
        return
    import concourse.bass_utils as _bu

    _orig = _bu.run_command

    def _patched(argv, **kwargs):
        argv = [
            "--enable-ldw-opt=true" if a == "--enable-ldw-opt=false" else a
            for a in argv
        ]
        return _orig(argv, **kwargs)

    _bu.run_command = _patched
    _cache["ldwopt"] = True


def _build_v2(use_dve=True, use_fp8=True):
    import concourse.mybir as mybir
    import concourse.tile as tile
    from concourse import bacc

    _maybe_enable_ldw_opt()

    op = _ensure_dve_op() if use_dve else None
    f32 = mybir.dt.float32
    bf16 = mybir.dt.bfloat16
    fp8 = mybir.dt.float8e3 if use_fp8 else mybir.dt.bfloat16
    SQRT = mybir.ActivationFunctionType.Sqrt

    nc = bacc.Bacc(
        "TRN2", target_bir_lowering=False, debug=False, num_devices=N_CORES
    )
    bt = nc.dram_tensor("bt", [128, HALF], fp8, kind="ExternalInput").ap()
    ewc = nc.dram_tensor("ewc", [128, EWC_COLS], f32, kind="ExternalInput").ap()
    o = nc.dram_tensor("o", [128, HALF], bf16, kind="ExternalOutput").ap()

    engs = lambda name: {
        "sync": nc.sync, "scalar": nc.scalar,
        "gpsimd": nc.gpsimd, "vector": nc.vector,
    }[name]

    PW = 2 * NBLK  # 1024-col processing pair
    with tile.TileContext(nc) as tc:
        with (
            tc.tile_pool(name="singles", bufs=1) as singles,
            tc.tile_pool(name="bt_pool", bufs=1) as bt_pool,
            tc.tile_pool(name="out_pool", bufs=10) as out_pool,
            tc.tile_pool(name="psum", bufs=4, space="PSUM") as psum,
        ):
            # --- input streams, all doorbells rung up front ---------------
            # ewc first on the scalar queue (small, needed by ~10us for
            # the stationaries); bank pairs spread over all three queues.
            ewc2 = singles.tile([128, EWC_COLS], f32)
            nc.scalar.dma_start(out=ewc2, in_=ewc)

            # Preload the Sqrt ACT table with the production signature.
            zt = singles.tile([128, 1], f32)
            nc.vector.memset(zt, 0.0)
            warm = singles.tile([128, 1], f32)
            nc.scalar.activation(out=warm, in_=zt, func=SQRT, bias=zt, scale=1.0)

            regions = []  # (start, width, tile)
            for eng_name, c0, w in IN_DMA:
                gt = bt_pool.tile([128, w], fp8, name=f"btr{c0}", tag=f"btr{c0}")
                regions.append((c0, w, gt))
                engs(eng_name).dma_start(out=gt, in_=bt[:, c0 : c0 + w])

            def rslice(c0, w):
                for r0, rw, gt in regions:
                    if r0 <= c0 and c0 + w <= r0 + rw:
                        return gt[:, c0 - r0 : c0 - r0 + w]
                raise AssertionError(f"unit [{c0},{c0 + w}) not in one region")

            # --- PE warm-up against the p-state ramp (vector memsets
            # first; no engine can write SBUF before ~7.2us anyway).
            # Warm-ups borrow a main psum buffer (8 banks total: 4 bufs
            # x 1024 f32); the first reuser only waits PE program order.
            dummy_w = singles.tile([128, 128], bf16)
            nc.vector.memset(dummy_w, 0.0)
            dummy_r = singles.tile([128, WARM_W], bf16)
            nc.vector.memset(dummy_r, 0.0)
            ps_warm = psum.tile([128, WARM_W], f32, tag="ps")
            for _ in range(N_WARM):
                nc.tensor.matmul(
                    ps_warm, lhsT=dummy_w, rhs=dummy_r, start=True, stop=True
                )

            # Two block-diagonal stationaries (pure cast-copies on the
            # otherwise-idle vector engine): em_q = -2*embT for the
            # exact-sqrt ACT units, em_k = -embT*k/4 for the DVE units.
            em_q = singles.tile([128, 128], bf16)
            em_k = singles.tile([128, 128], bf16)
            nc.vector.memset(em_q, 0.0)
            nc.vector.memset(em_k, 0.0)
            nc.vector.tensor_copy(out=em_q[0:64, 0:64], in_=ewc2[0:64, 0:DIM])
            nc.vector.tensor_copy(
                out=em_q[64:128, 64:128], in_=ewc2[64:128, DIM:128]
            )
            nc.vector.tensor_copy(
                out=em_k[0:64, 0:64], in_=ewc2[0:64, 128 : 128 + DIM]
            )
            nc.vector.tensor_copy(
                out=em_k[64:128, 64:128], in_=ewc2[64:128, 128 + DIM : 256]
            )

            bias_act = ewc2[:, 256:257]
            bias_T = ewc2[:, 257:258]
            a_t = ewc2[:, 258:259]
            c_t = ewc2[:, 259:260]

            # --- main pipeline --------------------------------------------
            for c0, w, typ, out_eng in UNITS:
                if not use_dve:
                    typ = "act"
                em = em_q if typ == "act" else em_k
                ps = psum.tile([128, w], f32, tag="ps")
                for j in range(w // NBLK):
                    nc.tensor.matmul(
                        ps[:, j * NBLK : (j + 1) * NBLK], lhsT=em,
                        rhs=rslice(c0 + j * NBLK, NBLK),
                        start=True, stop=True,
                    )
                out_t = out_pool.tile([128, w], bf16, tag="out")
                if typ == "act":
                    nc.scalar.activation(
                        out=out_t, in_=ps, func=SQRT, bias=bias_act, scale=1.0
                    )
                else:
                    nc.vector._custom_dve(
                        op, out=out_t, in0=ps,
                        in1=c_t.broadcast_to([128, w]),
                        s0=bias_T, s1=a_t,
                    )
                engs(out_eng).dma_start(out=o[:, c0 : c0 + w], in_=out_t)

    nc.compile()
    return nc


# ---------------------------------------------------------------------------
# Fallback (arbitrary bank norms / pathological query ranges): the v1
# bf16 all-ACT kernel, correct for any inputs. Never hit for MemoryBank
# data; kept for robustness.

FB_CHUNKS = [512, 512, 1024, 1024, 1024, 1024, 1024, 1024, 512, 512]
FB_GROUPS = [
    ("sync", [0]),
    ("scalar", [1, 2]),
    ("sync", [3, 4]),
    ("scalar", [5, 6]),
    ("scalar", [7, 8, 9]),
]
FB_ORDER = [1, 2, 0, 5, 6, 3, 4, 7, 8, 9]


def _build_fallback():
    import concourse.mybir as mybir
    import concourse.tile as tile
    from concourse import bacc

    f32 = mybir.dt.float32
    bf16 = mybir.dt.bfloat16
    SQRT = mybir.ActivationFunctionType.Sqrt

    offs = np.concatenate([[0], np.cumsum(FB_CHUNKS)])
    assert offs[-1] == HALF

    nc = bacc.Bacc(
        "TRN2", target_bir_lowering=False, debug=False, num_devices=N_CORES
    )
    bt = nc.dram_tensor("bt", [128, HALF], bf16, kind="ExternalInput").ap()
    ewc = nc.dram_tensor("ewc", [128, 192], f32, kind="ExternalInput").ap()
    o = nc.dram_tensor("o", [128, HALF], bf16, kind="ExternalOutput").ap()

    with tile.TileContext(nc) as tc:
        with (
            tc.tile_pool(name="singles", bufs=1) as singles,
            tc.tile_pool(name="bt_pool", bufs=1) as bt_pool,
            tc.tile_pool(name="sq_pool", bufs=3) as sq_pool,
            tc.tile_pool(name="out_pool", bufs=6) as out_pool,
            tc.tile_pool(name="psum", bufs=4, space="PSUM") as psum,
        ):
            ewc2 = singles.tile([128, 192], f32)
            nc.sync.dma_start(out=ewc2, in_=ewc)

            zt = singles.tile([128, 1], f32)
            nc.vector.memset(zt, 0.0)
            warm = singles.tile([128, 1], f32)
            nc.scalar.activation(out=warm, in_=zt, func=SQRT, bias=zt, scale=1.0)

            bts = {}
            for gi, (eng_name, cis) in enumerate(FB_GROUPS):
                lo, hi = int(offs[cis[0]]), int(offs[cis[-1] + 1])
                gt = bt_pool.tile(
                    [128, hi - lo], bf16, name=f"btg{gi}", tag=f"btg{gi}"
                )
                for ci in cis:
                    bts[ci] = gt[:, int(offs[ci]) - lo : int(offs[ci + 1]) - lo]
                eng = {"sync": nc.sync, "scalar": nc.scalar,
                       "gpsimd": nc.gpsimd}[eng_name]
                eng.dma_start(out=gt, in_=bt[:, lo:hi])

            dummy_w = singles.tile([128, 128], bf16)
            nc.gpsimd.memset(dummy_w, 0.0)
            dummy_r = singles.tile([128, 512], bf16)
            nc.gpsimd.memset(dummy_r, 0.0)
            ps_warm = psum.tile([128, 1024], f32, tag="ps")
            for _ in range(3):
                nc.tensor.matmul(
                    ps_warm[:, 0:512], lhsT=dummy_w, rhs=dummy_r,
                    start=True, stop=True,
                )

            em2bd_f = singles.tile([128, 128], f32)
            nc.vector.memset(em2bd_f, 0.0)
            nc.vector.tensor_scalar_mul(
                em2bd_f[0:64, 0:64], ewc2[0:64, 0:DIM], -2.0
            )
            nc.vector.tensor_scalar_mul(
                em2bd_f[64:128, 64:128], ewc2[64:128, 0:DIM], -2.0
            )
            em2bd = singles.tile([128, 128], bf16)
            nc.vector.tensor_copy(out=em2bd, in_=em2bd_f)

            sq_ewt = singles.tile([128, DIM], f32)
            nc.vector.tensor_mul(sq_ewt, ewc2[:, 128:192], ewc2[:, 128:192])
            bias = singles.tile([128, 1], f32)
            nc.vector.tensor_reduce(
                out=bias, in_=sq_ewt,
                axis=mybir.AxisListType.X, op=mybir.AluOpType.add,
            )

            onesbd_f = singles.tile([128, 128], f32)
            nc.vector.memset(onesbd_f, 0.0)
            nc.vector.memset(onesbd_f[0:64, 0:64], 1.0)
            nc.vector.memset(onesbd_f[64:128, 64:128], 1.0)
            onesbd = singles.tile([128, 128], bf16)
            nc.vector.tensor_copy(out=onesbd, in_=onesbd_f)

            for oi, ci in enumerate(FB_ORDER):
                w = FB_CHUNKS[ci]
                bt_c = bts[ci]
                ps = psum.tile([128, w], f32, tag="ps")
                for j in range(w // NBLK):
                    sl = slice(j * NBLK, (j + 1) * NBLK)
                    nc.tensor.matmul(
                        ps[:, sl], lhsT=em2bd, rhs=bt_c[:, sl],
                        start=True, stop=False,
                    )
                sq_c = sq_pool.tile([128, w], bf16, tag="sq")
                nc.vector.tensor_mul(sq_c, bt_c, bt_c)
                for j in range(w // NBLK):
                    sl = slice(j * NBLK, (j + 1) * NBLK)
                    nc.tensor.matmul(
                        ps[:, sl], lhsT=onesbd, rhs=sq_c[:, sl],
                        start=False, stop=True,
                    )
                cs = slice(int(offs[ci]), int(offs[ci + 1]))
                out_c = out_pool.tile([128, w], bf16, tag="out")
                nc.scalar.activation(
                    out=out_c, in_=ps, func=SQRT, bias=bias, scale=1.0
                )
                if oi == len(FB_ORDER) - 1:
                    nc.scalar.dma_start(out=o[:, cs], in_=out_c)
                elif oi % 2 == 0:
                    nc.gpsimd.dma_start(out=o[:, cs], in_=out_c)
                else:
                    nc.sync.dma_start(out=o[:, cs], in_=out_c)

    nc.compile()
    return nc


def _get_nc(which):
    import os

    use_dve = os.environ.get("KV2_USE_DVE", "1") == "1"
    use_fp8 = os.environ.get("KV2_USE_FP8", "1") == "1"
    key = ("nc", which, use_dve, use_fp8)
    if key not in _cache:
        _cache[key] = (
            _build_v2(use_dve, use_fp8) if which == "v2" else _build_fallback()
        )
    return _cache[key]


def _query_constants(emb_batch):
    """Per-query sqrt-fit constants. Returns None if out of the grid's
    domain (caller falls back)."""
    s = (emb_batch.astype(np.float64) ** 2).sum(1)
    sq = np.sqrt(s)
    if sq.min() <= 1.25:
        return None
    lo = (sq - 1.0) ** 2
    hi = (sq + 1.0) ** 2
    rho = hi / lo
    if rho.max() > RHO_GRID[-1, 0]:
        return None
    idx = np.searchsorted(RHO_GRID[:, 0], rho)
    A, Ch, Dh = (RHO_GRID[idx, j] for j in (1, 2, 3))
    S = (np.sqrt(lo) * Dh) ** 0.2
    a_t = S * A
    c_t = np.sqrt(lo) * Ch / (S * S)
    k = S / lo
    return {
        "k": k,
        "a": a_t,
        "c": c_t,
        "bias_T": k * (s + 1.0),
        "scale_act": 1.0 / k,
        "bias_act": s + 1.0,
    }


def _prep_v2(emb_batch, bank, qc):
    import os

    import ml_dtypes

    e3m4 = (
        ml_dtypes.float8_e3m4
        if os.environ.get("KV2_USE_FP8", "1") == "1"
        else ml_dtypes.bfloat16
    )
    emb_batch = np.asarray(emb_batch, dtype=np.float32)
    bank = np.asarray(bank, dtype=np.float32)

    # ACT stationary Wq[d, m] = -2*emb[m, d] / 8 (x8 bank scale undone);
    # DVE stationary Wk[d, m] = -emb[m, d] * k_m / 4 (x8 scale and the
    # fit's T = k*dist^2 normalization folded in).
    Wq = (emb_batch.T * np.float32(-0.25)).astype(np.float32)
    Wk = (emb_batch.T * (-(qc["k"] / 4.0))[None, :]).astype(np.float32)
    ewc_host = np.zeros((128, EWC_COLS), dtype=np.float32)
    ewc_host[:, 0:128] = np.tile(Wq, (2, 2))
    ewc_host[:, 128:256] = np.tile(Wk, (2, 2))
    for col, key in (
        (256, "bias_act"), (257, "bias_T"), (258, "a"), (259, "c"),
    ):
        ewc_host[:, col] = np.tile(qc[key].astype(np.float32), 2)

    bankT8 = np.ascontiguousarray(bank.T * np.float32(8.0)).astype(e3m4)
    in_maps = []
    for c in range(N_CORES):
        sh = bankT8[:, c * SHARD : (c + 1) * SHARD]
        btc = np.ascontiguousarray(
            np.concatenate([sh[:, :HALF], sh[:, HALF:]], axis=0)
        )
        in_maps.append({"bt": btc, "ewc": ewc_host})
    return in_maps


def _prep_fallback(emb_batch, bank):
    import ml_dtypes

    bf16 = ml_dtypes.bfloat16
    emb_batch = np.asarray(emb_batch, dtype=np.float32)
    bank = np.asarray(bank, dtype=np.float32)
    ewc_host = np.ascontiguousarray(
        np.concatenate(
            [np.tile(emb_batch.T, (2, 2)), np.tile(emb_batch, (2, 1))], axis=1
        )
    )
    bankT = bank.T
    in_maps = []
    for c in range(N_CORES):
        sh = bankT[:, c * SHARD : (c + 1) * SHARD]
        btc = np.ascontiguousarray(
            np.concatenate([sh[:, :HALF], sh[:, HALF:]], axis=0)
        ).astype(bf16)
        in_maps.append({"bt": btc, "ewc": ewc_host})
    return in_maps


def kernel(emb_batch, bank):
    global last_run
    from concourse.bass_utils import run_bass_kernel_spmd

    emb_batch = np.asarray(emb_batch, dtype=np.float32)
    bank = np.asarray(bank, dtype=np.float32)
    # Read-only validation: MemoryBank rows are L2-normalized and the
    # per-query dist^2 ranges must sit inside the fit grid.
    norms = np.einsum("bd,bd->b", bank, bank)
    qc = None
    if np.abs(norms - 1.0).max() < 1e-3:
        qc = _query_constants(emb_batch)

    if qc is not None:
        nc = _get_nc("v2")
        in_maps = _prep_v2(emb_batch, bank, qc)
    else:
        nc = _get_nc("fb")
        in_maps = _prep_fallback(emb_batch, bank)

    res = run_bass_kernel_spmd(nc, in_maps, core_ids=list(range(N_CORES)))
    last_run = res
    out = np.empty((BATCH, BANK), dtype=np.float32)
    for c in range(N_CORES):
        oc = res.results[c]["o"]  # [128, HALF] bf16: rows (h*64 + n)
        oc = np.asarray(oc).astype(np.float32)
        out[:, c * SHARD : c * SHARD + HALF] = oc[0:64]
        out[:, c * SHARD + HALF : (c + 1) * SHARD] = oc[64:128]
    return out
